# revision 30
# baseline (speedup 1.0000x reference)
"""MixHop (2-hop) GNN forward on 8 TRN2 NeuronCores.

Sharding: adj and the output are row-sharded over N=8192 across 8 cores
(1024 rows each); x and all weights are replicated. Each propagation
adj_loc @ v is a local [1024,8192]@[8192,F] matmul; v is produced
row-sharded and AllGathered between hops.

Precision: propagation matmuls (adj-sided, the bulk of bytes+flops) run
in bf16 with fp32 PSUM accumulation; dense-layer transforms and BN run
in fp32r (full-rate reduced fp32). Measured end-to-end relative error
~2e-3.

Orientation notes:
- "natural"   = rows on partitions (needed for AllGather row-concat and
  as the K axis of the next propagation)
- "transposed" = features on partitions (needed as lhsT of the next
  dense layer; makes BatchNorm affine per-partition)
Pass B and D emit transposed outputs directly; pass A's t1 half and
pass C's s1 half are transposed on the PE with an identity matmul.
b1[0] (hop-0 bias of layer 1) is dropped: a per-column constant shift
is exactly cancelled by the training-mode BatchNorm that follows.

Scheduling notes:
- Every AllGather is split into two row-half chunks. Producer passes
  emit their first row-half, trigger chunk-a, and compute the second
  half under it; consumer passes contract chunk-a's rows while chunk-b
  is still gathering. To keep the contraction k-axis contiguous per
  chunk, the host permutes adj's columns (and x's rows) into
  "half-major" order: [r0 rows0:512 | r1 rows0:512 | ... | r0 rows
  512:1024 | ...]. Local row order (outputs) is unpermuted.
- Propagation k-loops process 4 k-slabs per iteration -> ~4.3us
  contiguous matmul bursts, which hold the PE HAM clock-gate at full
  rate (short bursts leave the PE throttled to 1.2 GHz).
- BN stats/AllReduce/normalize for h tiles 0..7 run under pass B; T2's
  first 8 k-tiles accumulate under AllReduce-b so only a short tail
  waits on it. y0.T runs under AllGather2b; s1 transposes run before
  AllGather3.
- DMA loads alternate between the two HWDGE rings (sync/scalar);
  SBUF->DRAM drains go via SWDGE (gpsimd).
"""
import sys
from contextlib import ExitStack

sys.path.insert(0, "/opt/trn_rl_repo")

import numpy as np

N, IN, H, OUT = 8192, 128, 512, 256
NC = 8
ROWS = N // NC          # 1024 rows per core
KT = N // 128           # 64 k-tiles of the propagation contraction
KH = KT // 2            # 32 k-tiles per gather chunk
HT = 3 * H // 128       # 12 feature tiles of h.T
H2T = 3 * OUT // 128    # 6 feature tiles of h2.T
EPS = 1e-5

_BUILT = {}


def build_program():
    """Build and compile the Bass program (cached)."""
    if "nc" in _BUILT:
        return _BUILT["nc"]

    import concourse.bacc as bacc
    import concourse.tile as tile
    import concourse.mybir as mybir
    from concourse.alu_op_type import AluOpType

    f32 = mybir.dt.float32
    f32r = mybir.dt.float32r
    bf16 = mybir.dt.bfloat16
    AF = mybir.ActivationFunctionType
    AX = mybir.AxisListType

    nc = bacc.Bacc("TRN2", target_bir_lowering=False, debug=False,
                   num_devices=NC)

    import os as _os
    if _os.environ.get("LDW_EXPERIMENT"):
        # harmless extra input to change the program hash (cache bust)
        nc.dram_tensor("cachebust", [1, 4], mybir.dt.float32,
                       kind="ExternalInput")

    # ---- external inputs (per-core values supplied by the host) ----
    adjT_d = nc.dram_tensor("adjT", [N, ROWS], bf16, kind="ExternalInput")
    xT_d = nc.dram_tensor("xT", [IN, N], f32r, kind="ExternalInput")
    xTloc_d = nc.dram_tensor("xTloc", [IN, ROWS], f32r, kind="ExternalInput")
    w1cat_d = nc.dram_tensor("w1cat", [IN, 2 * H], f32r, kind="ExternalInput")
    w1h0_d = nc.dram_tensor("w1h0", [IN, H], f32r, kind="ExternalInput")
    b1bc_d = nc.dram_tensor("b1bc", [128, 2 * H], f32, kind="ExternalInput")
    w2cat_d = nc.dram_tensor("w2cat", [3 * H, 2 * OUT], f32r, kind="ExternalInput")
    w2h0_d = nc.dram_tensor("w2h0", [3 * H, OUT], f32r, kind="ExternalInput")
    b2bc_d = nc.dram_tensor("b2bc", [128, 2 * OUT], f32, kind="ExternalInput")
    b2h0T_d = nc.dram_tensor("b2h0T", [128, 2], f32, kind="ExternalInput")
    wf_d = nc.dram_tensor("wf", [3 * OUT, OUT], f32r, kind="ExternalInput")
    bfT_d = nc.dram_tensor("bfT", [128, 2], f32, kind="ExternalInput")
    gcol_d = nc.dram_tensor("gcol", [128, HT], f32, kind="ExternalInput")
    bcol_d = nc.dram_tensor("bcol", [128, HT], f32, kind="ExternalInput")
    ident_d = nc.dram_tensor("ident", [128, 128], f32, kind="ExternalInput")

    outT_d = nc.dram_tensor("outT", [OUT, ROWS], f32, kind="ExternalOutput")

    rg = [list(range(NC))]

    def ring(k):
        return nc.sync if k % 2 == 0 else nc.scalar

    with tile.TileContext(nc) as tc, ExitStack() as st:
        dram = st.enter_context(tc.tile_pool(name="dram", bufs=1, space="DRAM"))
        P = st.enter_context(tc.tile_pool(name="persist", bufs=1))
        DR = st.enter_context(tc.tile_pool(name="drain", bufs=2))

        # ---- DRAM intermediates ----
        v_dram = dram.tile([N, H], bf16, name="v_dram")   # x1 transform only
        agi = {}
        ago = {}
        for nm, rows, cols in (("ag1", ROWS // 2, H), ("ag2", ROWS // 2,
                                                       2 * OUT),
                               ("ag3", ROWS // 2, OUT)):
            for ch in "ab":
                agi[nm + ch] = dram.tile([rows, cols], bf16,
                                         name=f"{nm}{ch}_in")
                ago[nm + ch] = dram.tile([rows * NC, cols], bf16,
                                         name=f"{nm}{ch}_out",
                                         addr_space="Shared")
        ar_a_in = dram.tile([128, 16], f32, name="ar_a_in")
        ar_a_out = dram.tile([128, 16], f32, name="ar_a_out",
                             addr_space="Shared")
        ar_b_in = dram.tile([128, 8], f32, name="ar_b_in")
        ar_b_out = dram.tile([128, 8], f32, name="ar_b_out",
                             addr_space="Shared")

        def gather(nm, ch):
            nc.gpsimd.collective_compute(
                "AllGather", AluOpType.bypass, replica_groups=rg,
                ins=[agi[nm + ch][:].opt()], outs=[ago[nm + ch][:].opt()])

        def gsrc(nm, k):
            """k-slab [128, cols] of the gathered tensor, half-major order."""
            t = ago[nm + ("a" if k < KH else "b")]
            kk = k if k < KH else k - KH
            return t[kk * 128:(kk + 1) * 128, :]

        def gsrc_pair(nm, q):
            """Pair-slab AP [128, 2, cols] for k-slabs 2q, 2q+1."""
            t = ago[nm + ("a" if 2 * q < KH else "b")]
            qq = q if 2 * q < KH else q - KH // 2
            return t[:].rearrange("(a two p) f -> a p two f",
                                  two=2, p=128)[qq]

        # ---- small persistents (to the end) ----
        # w1cat first: it gates the very first matmul of the kernel
        W1P = st.enter_context(tc.tile_pool(name="w1pool", bufs=1))
        w1cat_sb = W1P.tile([IN, 2 * H], f32r, name="w1cat_sb")
        nc.scalar.dma_start(w1cat_sb[:], w1cat_d[:, :])
        b1bc_sb = W1P.tile([128, 2 * H], f32, name="b1bc_sb")
        nc.scalar.dma_start(b1bc_sb[:], b1bc_d[:, :])
        xTloc_sb = P.tile([IN, ROWS], f32r, name="xTloc_sb")
        nc.scalar.dma_start(xTloc_sb[:], xTloc_d[:, :])
        w1h0_sb = P.tile([IN, H], f32r, name="w1h0_sb")
        nc.scalar.dma_start(w1h0_sb[:], w1h0_d[:, :])
        b2h0T_sb = P.tile([128, 2], f32, name="b2h0T_sb")
        nc.scalar.dma_start(b2h0T_sb[:], b2h0T_d[:, :])
        bfT_sb = P.tile([128, 2], f32, name="bfT_sb")
        nc.scalar.dma_start(bfT_sb[:], bfT_d[:, :])
        gcol_sb = P.tile([128, HT], f32, name="gcol_sb")
        nc.scalar.dma_start(gcol_sb[:], gcol_d[:, :])
        bcol_sb = P.tile([128, HT], f32, name="bcol_sb")
        nc.scalar.dma_start(bcol_sb[:], bcol_d[:, :])
        ident_sb = P.tile([128, 128], f32, name="ident_sb")
        nc.scalar.dma_start(ident_sb[:], ident_d[:, :])
        wf_sb = [P.tile([128, OUT], f32r, name=f"wf{k}") for k in range(H2T)]
        for k in range(H2T):
            nc.scalar.dma_start(wf_sb[k][:], wf_d[k * 128:(k + 1) * 128, :])
        sumc = P.tile([128, HT], f32, name="sumc")
        sqc = P.tile([128, HT], f32, name="sqc")
        scale_c = P.tile([128, HT], f32, name="scale_c")
        shift_c = P.tile([128, HT], f32, name="shift_c")
        stat_a = P.tile([128, 16], f32, name="stat_a")
        stat_b = P.tile([128, 8], f32, name="stat_b")
        # h2.T (fp32r): tiles 0-1 y0.T, 2-3 s1.T, 4-5 s2b.T
        h2T = [P.tile([128, ROWS], f32r, name=f"h2T{t}") for t in range(H2T)]

        def bn_affine(stat, lo, hi):
            """Compute scale/shift columns [lo,hi) from gathered stats."""
            w = hi - lo
            mu = DR.tile([128, w], f32, name="mu", tag=f"mu{lo}")
            nc.vector.tensor_scalar_mul(mu[:], stat[:, :w], 1.0 / N)
            ex2 = DR.tile([128, w], f32, name="ex2", tag=f"ex2{lo}")
            nc.vector.tensor_scalar(ex2[:], stat[:, w:2 * w], 1.0 / N, EPS,
                                    AluOpType.mult, AluOpType.add)
            var = DR.tile([128, w], f32, name="var", tag=f"var{lo}")
            nc.vector.scalar_tensor_tensor(var[:], mu[:], -1.0, mu[:],
                                           AluOpType.mult, AluOpType.mult)
            nc.vector.tensor_add(var[:], var[:], ex2[:])
            std = DR.tile([128, w], f32, name="std", tag=f"std{lo}")
            nc.scalar.activation(std[:], var[:], AF.Sqrt)
            rstd = DR.tile([128, w], f32, name="rstd", tag=f"rstd{lo}")
            nc.vector.reciprocal(rstd[:], std[:])
            nc.vector.tensor_mul(scale_c[:, lo:hi], gcol_sb[:, lo:hi], rstd[:])
            nc.vector.scalar_tensor_tensor(shift_c[:, lo:hi], mu[:], -1.0,
                                           scale_c[:, lo:hi],
                                           AluOpType.mult, AluOpType.mult)
            nc.vector.tensor_add(shift_c[:, lo:hi], shift_c[:, lo:hi],
                                 bcol_sb[:, lo:hi])

        def stats_for(tiles, sq_pool, tag):
            for t in tiles:
                nc.vector.reduce_sum(sumc[:, t:t + 1], hT[t][:], axis=AX.X)
                sq = sq_pool.tile([128, ROWS], f32, name=tag, tag=tag)
                nc.vector.scalar_tensor_tensor(
                    sq[:], hT[t][:], 1.0, hT[t][:],
                    AluOpType.mult, AluOpType.mult,
                    accum_out=sqc[:, t:t + 1])

        # ============ T1 + pass A (t2a), pipelined ===========================
        with (
            tc.tile_pool(name="v2pool", bufs=1) as V2P,
            tc.tile_pool(name="xkpool", bufs=1) as XKP,
        ):
            v2sb = [V2P.tile([128, H], bf16, name=f"v2s{k}")
                    for k in range(KT)]
            xk_sb = [XKP.tile([128, 128], f32r, name=f"xk{k}")
                     for k in range(KT)]

            # T1-x2: v2 = x @ W1[2] + b1[2]  (SBUF-resident, bf16)
            with tc.tile_pool(name="t1ps2", bufs=3, space="PSUM") as T1PS2:
                for k in range(KT):
                    nc.gpsimd.dma_start(xk_sb[k][:],
                                        xT_d[:, k * 128:(k + 1) * 128])
                    vps2 = T1PS2.tile([128, H], f32, name="vps2", tag="vps2")
                    nc.tensor.matmul(vps2[:], xk_sb[k][:],
                                     w1cat_sb[:, H:2 * H],
                                     start=True, stop=True)
                    nc.vector.tensor_tensor(v2sb[k][:], vps2[:],
                                            b1bc_sb[:, H:2 * H],
                                            AluOpType.add)

            # A-t2a group 0 with T1-x1 interleaved into its dense PE stream
            with (
                tc.tile_pool(name="ag0slab", bufs=2) as AS,
                tc.tile_pool(name="ag0ps", bufs=1, space="PSUM") as APS,
                tc.tile_pool(name="t1ps1", bufs=2, space="PSUM") as T1PS1,
            ):
                acc = [APS.tile([128, H], f32, name=f"acc0{m}",
                                tag=f"acc0{m}") for m in range(4)]
                for q in range(KT // 4):
                    slabs = []
                    for t in range(4):
                        k = 4 * q + t
                        asl = AS.tile([128, H], bf16, name=f"asl{t}",
                                      tag=f"asl{t}")
                        ring(t).dma_start(asl[:],
                                          adjT_d[k * 128:(k + 1) * 128, 0:H])
                        slabs.append(asl)
                    for t in range(4):
                        k = 4 * q + t
                        for m in range(4):
                            nc.tensor.matmul(
                                acc[m][:], slabs[t][:, m * 128:(m + 1) * 128],
                                v2sb[k][:], start=(k == 0),
                                stop=(k == KT - 1))
                    # T1-x1 for 4 slabs, slotted into the same stream
                    for t in range(4):
                        k = 4 * q + t
                        vps1 = T1PS1.tile([128, H], f32, name="vps1",
                                          tag="vps1")
                        nc.tensor.matmul(vps1[:], xk_sb[k][:],
                                         w1cat_sb[:, 0:H],
                                         start=True, stop=True)
                        v1sb = DR.tile([128, H], bf16, name="v1sb",
                                       tag="v1sb")
                        nc.vector.tensor_tensor(v1sb[:], vps1[:],
                                                b1bc_sb[:, 0:H],
                                                AluOpType.add)
                        nc.gpsimd.dma_start(
                            v_dram[k * 128:(k + 1) * 128, :], v1sb[:])
                for m in range(4):
                    t2a = DR.tile([128, H], bf16, name="t2a", tag="t2a")
                    nc.vector.tensor_copy(t2a[:], acc[m][:])
                    nc.gpsimd.dma_start(
                        agi["ag1a"][m * 128:(m + 1) * 128, :], t2a[:])

            gather("ag1", "a")

            # A-t2a group 1
            with (
                tc.tile_pool(name="ag1slab", bufs=2) as AS1,
                tc.tile_pool(name="ag1ps", bufs=1, space="PSUM") as APS1,
            ):
                acc1 = [APS1.tile([128, H], f32, name=f"acc1{m}",
                                  tag=f"acc1{m}") for m in range(4)]
                for q in range(KT // 4):
                    slabs = []
                    for t in range(4):
                        k = 4 * q + t
                        asl = AS1.tile([128, H], bf16, name=f"bsl{t}",
                                       tag=f"bsl{t}")
                        ring(t).dma_start(asl[:],
                                          adjT_d[k * 128:(k + 1) * 128,
                                                 H:ROWS])
                        slabs.append(asl)
                    for t in range(4):
                        k = 4 * q + t
                        for m in range(4):
                            nc.tensor.matmul(
                                acc1[m][:], slabs[t][:, m * 128:(m + 1) * 128],
                                v2sb[k][:], start=(k == 0),
                                stop=(k == KT - 1))
                for m in range(4):
                    t2a = DR.tile([128, H], bf16, name="t2a", tag="t2a")
                    nc.vector.tensor_copy(t2a[:], acc1[m][:])
                    nc.gpsimd.dma_start(
                        agi["ag1b"][m * 128:(m + 1) * 128, :], t2a[:])

            gather("ag1", "b")

        # hT in fp32r: tiles 0-3 hop0.T, 4-7 t1.T, 8-11 t2b.T.
        PH = st.enter_context(tc.tile_pool(name="hpool", bufs=1))
        hT = [PH.tile([128, ROWS], f32r, name=f"hT{t}") for t in range(HT)]

        # ========== hop0.T + its stats (runs under AllGather1) ===============
        with (
            tc.tile_pool(name="h0ps", bufs=2, space="PSUM") as H0PS,
            tc.tile_pool(name="sqps0", bufs=1, space="PSUM") as SQPS0,
        ):
            for mo in range(4):
                for n in range(2):
                    h0ps = H0PS.tile([128, H], f32, name="h0ps", tag="h0ps")
                    nc.tensor.matmul(h0ps[:],
                                     w1h0_sb[:, mo * 128:(mo + 1) * 128],
                                     xTloc_sb[:, n * H:(n + 1) * H],
                                     start=True, stop=True)
                    nc.vector.tensor_copy(hT[mo][:, n * H:(n + 1) * H],
                                          h0ps[:])
            stats_for(range(4), SQPS0, "sq0")

        # t1 natural (fp32), transposed as soon as each row-group lands
        PT1 = st.enter_context(tc.tile_pool(name="t1nat", bufs=1))
        t1_sb = [PT1.tile([128, H], f32, name=f"t1n{m}") for m in range(8)]

        # ========= A-t1 row-groups + transposes ==============================
        for g in range(2):
            with (
                tc.tile_pool(name=f"a2slab{g}", bufs=2) as AS2,
                tc.tile_pool(name=f"aps2{g}", bufs=1, space="PSUM") as APS2,
            ):
                acc2 = [APS2.tile([128, H], f32, name=f"ac2{g}{m}",
                                  tag=f"ac2{g}{m}") for m in range(4)]
                for q in range(KT // 4):
                    slabs = []
                    for t in range(4):
                        k = 4 * q + t
                        asl = AS2.tile([128, H], bf16, name=f"a2s{t}",
                                       tag=f"a2s{t}")
                        ring(t).dma_start(
                            asl[:], adjT_d[k * 128:(k + 1) * 128,
                                           g * H:(g + 1) * H])
                        slabs.append(asl)
                    v1p = AS2.tile([128, 2 * H], bf16, name="v1p", tag="v1p")
                    nc.scalar.dma_start(
                        v1p[:].rearrange("p (two f) -> p two f", two=2),
                        v_dram[:].rearrange("(a two p) f -> a p two f",
                                            two=2, p=128)[2 * q])
                    v1p2 = AS2.tile([128, 2 * H], bf16, name="v1p2",
                                    tag="v1p2")
                    nc.sync.dma_start(
                        v1p2[:].rearrange("p (two f) -> p two f", two=2),
                        v_dram[:].rearrange("(a two p) f -> a p two f",
                                            two=2, p=128)[2 * q + 1])
                    vv = [v1p[:, 0:H], v1p[:, H:2 * H],
                          v1p2[:, 0:H], v1p2[:, H:2 * H]]
                    for t in range(4):
                        k = 4 * q + t
                        for m in range(4):
                            nc.tensor.matmul(
                                acc2[m][:],
                                slabs[t][:, m * 128:(m + 1) * 128],
                                vv[t], start=(k == 0), stop=(k == KT - 1))
                for m in range(4):
                    nc.vector.tensor_copy(t1_sb[4 * g + m][:], acc2[m][:])
            with tc.tile_pool(name=f"tps{g}", bufs=4, space="PSUM") as TPS:
                for c in range(4):
                    for m in range(4 * g, 4 * g + 4):
                        tp = TPS.tile([128, 128], f32, name="tp", tag="tp")
                        nc.tensor.transpose(
                            tp[:], t1_sb[m][:, c * 128:(c + 1) * 128],
                            ident_sb[:])
                        nc.vector.tensor_copy(
                            hT[4 + c][:, m * 128:(m + 1) * 128], tp[:])

        # ========= t1 stats + AllReduce-a + normalize tiles 0..7 =============
        with tc.tile_pool(name="sqps1", bufs=1, space="PSUM") as SQPS1:
            stats_for(range(4, 8), SQPS1, "sq1")
        nc.gpsimd.dma_start(ar_a_in[:, 0:8], sumc[:, 0:8])
        nc.gpsimd.dma_start(ar_a_in[:, 8:16], sqc[:, 0:8])
        nc.gpsimd.collective_compute(
            "AllReduce", AluOpType.add, replica_groups=rg,
            ins=[ar_a_in[:].opt()], outs=[ar_a_out[:].opt()])
        nc.sync.dma_start(stat_a[:], ar_a_out[:, :])
        bn_affine(stat_a, 0, 8)
        for t in range(8):
            nc.scalar.activation(hT[t][:], hT[t][:], AF.Relu,
                                 bias=shift_c[:, t:t + 1],
                                 scale=scale_c[:, t:t + 1])

        # layer-2 weights, loaded during pass B
        PW2 = st.enter_context(tc.tile_pool(name="w2pool", bufs=1))
        w2cat_sb = [PW2.tile([128, 2 * OUT], f32r, name=f"w2cat{k}")
                    for k in range(HT)]
        for k in range(HT):
            nc.scalar.dma_start(w2cat_sb[k][:],
                                w2cat_d[k * 128:(k + 1) * 128, :])
        w2h0_sb = [PW2.tile([128, OUT], f32r, name=f"w2h0{k}")
                   for k in range(HT)]
        for k in range(HT):
            nc.scalar.dma_start(w2h0_sb[k][:],
                                w2h0_d[k * 128:(k + 1) * 128, :])
        b2bc_sb = PW2.tile([128, 2 * OUT], f32, name="b2bc_sb")
        nc.scalar.dma_start(b2bc_sb[:], b2bc_d[:, :])

        # ================= B: t2b.T = (adj_loc @ t2a_full).T =================
        with (
            tc.tile_pool(name="bslabs", bufs=2) as BS,
            tc.tile_pool(name="bps", bufs=1, space="PSUM") as BPS,
        ):
            psb = [BPS.tile([128, H], f32, name=f"psb{i}", tag=f"psb{i}")
                   for i in range(8)]  # i = mo*2+n
            for q in range(KT // 2):
                aslab = BS.tile([128, ROWS], bf16, name="aslab", tag="aslab")
                nc.sync.dma_start(aslab[:],
                                  adjT_d[2 * q * 128:(2 * q + 1) * 128, :])
                aslab2 = BS.tile([128, ROWS], bf16, name="aslab2",
                                 tag="aslab2")
                nc.scalar.dma_start(
                    aslab2[:], adjT_d[(2 * q + 1) * 128:(2 * q + 2) * 128, :])
                tsp = BS.tile([128, 2 * H], bf16, name="tsp", tag="tsp")
                nc.scalar.dma_start(
                    tsp[:].rearrange("p (two f) -> p two f", two=2),
                    gsrc_pair("ag1", q))
                for t, asl in ((0, aslab), (1, aslab2)):
                    k = 2 * q + t
                    for mo in range(4):
                        for n in range(2):
                            nc.tensor.matmul(
                                psb[mo * 2 + n][:],
                                tsp[:, t * H + mo * 128:
                                    t * H + (mo + 1) * 128],
                                asl[:, n * H:(n + 1) * H],
                                start=(k == 0), stop=(k == KT - 1))
            for mo in range(4):
                for n in range(2):
                    nc.vector.tensor_copy(hT[8 + mo][:, n * H:(n + 1) * H],
                                          psb[mo * 2 + n][:])

        # ========== stats for t2b + AllReduce-b ==============================
        with tc.tile_pool(name="sqps2", bufs=1, space="PSUM") as SQPS2:
            stats_for(range(8, HT), SQPS2, "sq2")
        nc.gpsimd.dma_start(ar_b_in[:, 0:4], sumc[:, 8:12])
        nc.gpsimd.dma_start(ar_b_in[:, 4:8], sqc[:, 8:12])
        nc.gpsimd.collective_compute(
            "AllReduce", AluOpType.add, replica_groups=rg,
            ins=[ar_b_in[:].opt()], outs=[ar_b_out[:].opt()])

        # ========== T2 phase 1 (k=0..7) under AllReduce-b ====================
        with tc.tile_pool(name="ypsb", bufs=1, space="PSUM") as YPSb:
            ypss = [None] * 8
            for m in range(4, 8):
                ypss[m] = YPSb.tile([128, 2 * OUT], f32, name=f"ypsb{m}",
                                    tag=f"ypsb{m}")

            def t2_phase2(half):
                for m in range(4 * half, 4 * half + 4):
                    for k in range(8, HT):
                        nc.tensor.matmul(ypss[m][:],
                                         hT[k][:, m * 128:(m + 1) * 128],
                                         w2cat_sb[k][:],
                                         start=False, stop=(k == HT - 1))
                    ysb = DR.tile([128, 2 * OUT], bf16, name="ysb", tag="ysb")
                    nc.vector.tensor_tensor(ysb[:], ypss[m][:], b2bc_sb[:],
                                            AluOpType.add)
                    nc.gpsimd.dma_start(
                        agi["ag2" + "ab"[half]][
                            (m - 4 * half) * 128:(m - 4 * half + 1) * 128, :],
                        ysb[:])
                gather("ag2", "ab"[half])

            with tc.tile_pool(name="ypsa", bufs=1, space="PSUM") as YPSa:
                for m in range(4):
                    ypss[m] = YPSa.tile([128, 2 * OUT], f32, name=f"ypsa{m}",
                                        tag=f"ypsa{m}")
                for m in range(8):
                    for k in range(8):
                        nc.tensor.matmul(ypss[m][:],
                                         hT[k][:, m * 128:(m + 1) * 128],
                                         w2cat_sb[k][:],
                                         start=(k == 0), stop=False)

                # AllReduce-b lands: finish BN for t2b tiles
                nc.sync.dma_start(stat_b[:], ar_b_out[:, :])
                bn_affine(stat_b, 8, HT)
                for t in range(8, HT):
                    nc.scalar.activation(hT[t][:], hT[t][:], AF.Relu,
                                         bias=shift_c[:, t:t + 1],
                                         scale=scale_c[:, t:t + 1])
                t2_phase2(0)

            # YPSa closed: its banks free for y0T, which fills the PE while
            # AllGather2a runs; phase 2b follows.
            with tc.tile_pool(name="y0ps", bufs=2, space="PSUM") as Y0PS:
                for mo in range(2):
                    for n in range(2):
                        y0ps = Y0PS.tile([128, H], f32, name="y0ps",
                                         tag="y0ps")
                        for k in range(HT):
                            nc.tensor.matmul(
                                y0ps[:],
                                w2h0_sb[k][:, mo * 128:(mo + 1) * 128],
                                hT[k][:, n * H:(n + 1) * H],
                                start=(k == 0), stop=(k == HT - 1))
                        nc.vector.tensor_scalar_add(
                            h2T[mo][:, n * H:(n + 1) * H], y0ps[:],
                            b2h0T_sb[:, mo:mo + 1])
                t2_phase2(1)

        # s1 natural (fp32), transposed before AllGather3 completes
        PS1 = st.enter_context(tc.tile_pool(name="s1nat", bufs=1))
        s1_sb = [PS1.tile([128, OUT], f32, name=f"s1n{m}") for m in range(8)]

        # ========== C: [s1|s2a] = adj_loc @ [y1|y2] (natural) ================
        # Two row-groups (disjoint adjT column halves) so AllGather3a fires
        # after group 0 and pass D's first half overlaps group 1 (4+4 banks).
        def c_group(g, CS, CPS):
            psc = [CPS.tile([128, 2 * OUT], f32, name=f"psc{g}{m}",
                            tag=f"psc{g}{m}") for m in range(4)]
            for q in range(KT // 2):
                aslab = CS.tile([128, H], bf16, name="aslab", tag="aslab")
                nc.sync.dma_start(
                    aslab[:], adjT_d[2 * q * 128:(2 * q + 1) * 128,
                                     g * H:(g + 1) * H])
                aslab2 = CS.tile([128, H], bf16, name="aslab2", tag="aslab2")
                nc.scalar.dma_start(
                    aslab2[:], adjT_d[(2 * q + 1) * 128:(2 * q + 2) * 128,
                                      g * H:(g + 1) * H])
                ysp = CS.tile([128, 4 * OUT], bf16, name="ysp", tag="ysp")
                nc.sync.dma_start(
                    ysp[:].rearrange("p (two f) -> p two f", two=2),
                    gsrc_pair("ag2", q))
                for t, asl in ((0, aslab), (1, aslab2)):
                    for m in range(4):
                        nc.tensor.matmul(
                            psc[m][:], asl[:, m * 128:(m + 1) * 128],
                            ysp[:, t * 2 * OUT:(t + 1) * 2 * OUT],
                            start=(2 * q + t == 0),
                            stop=(2 * q + t == KT - 1))
            for m in range(4):
                gm = 4 * g + m
                nc.vector.tensor_copy(s1_sb[gm][:], psc[m][:, :OUT])
                s2a = DR.tile([128, OUT], bf16, name="s2a", tag="s2a")
                nc.vector.tensor_copy(s2a[:], psc[m][:, OUT:])
                nc.gpsimd.dma_start(
                    agi["ag3" + "ab"[g]][m * 128:(m + 1) * 128, :], s2a[:])

        with (
            tc.tile_pool(name="cslabs0", bufs=2) as CS0,
            tc.tile_pool(name="cps0", bufs=1, space="PSUM") as CPS0,
        ):
            c_group(0, CS0, CPS0)
        gather("ag3", "a")

        # C group 1 runs while AllGather3a is in flight; pass D's first half
        # (k-slabs 0..31, fed by AllGather3a) shares the PE with it.
        with (
            tc.tile_pool(name="dslabs", bufs=2) as DS,
            tc.tile_pool(name="dps", bufs=1, space="PSUM") as DPS,
        ):
            psd = [DPS.tile([128, H], f32, name=f"psd{i}", tag=f"psd{i}")
                   for i in range(4)]  # i = mo*2+n

            def d_quads(q0, q1):
                for q in range(q0, q1):
                    slabs = []
                    for t in range(4):
                        k = 4 * q + t
                        asl = DS.tile([128, ROWS], bf16, name=f"dsl{t}",
                                      tag=f"dsl{t}")
                        ring(t).dma_start(asl[:],
                                          adjT_d[k * 128:(k + 1) * 128, :])
                        slabs.append(asl)
                    sp1 = DS.tile([128, 2 * OUT], bf16, name="sp1", tag="sp1")
                    nc.sync.dma_start(
                        sp1[:].rearrange("p (two f) -> p two f", two=2),
                        gsrc_pair("ag3", 2 * q))
                    sp2 = DS.tile([128, 2 * OUT], bf16, name="sp2", tag="sp2")
                    nc.scalar.dma_start(
                        sp2[:].rearrange("p (two f) -> p two f", two=2),
                        gsrc_pair("ag3", 2 * q + 1))
                    ss = [sp1[:, 0:OUT], sp1[:, OUT:2 * OUT],
                          sp2[:, 0:OUT], sp2[:, OUT:2 * OUT]]
                    for t in range(4):
                        k = 4 * q + t
                        for mo in range(2):
                            for n in range(2):
                                nc.tensor.matmul(
                                    psd[mo * 2 + n][:],
                                    ss[t][:, mo * 128:(mo + 1) * 128],
                                    slabs[t][:, n * H:(n + 1) * H],
                                    start=(k == 0), stop=(k == KT - 1))

            with (
                tc.tile_pool(name="cslabs1", bufs=2) as CS1,
                tc.tile_pool(name="cps1", bufs=1, space="PSUM") as CPS1,
            ):
                c_group(1, CS1, CPS1)
                d_quads(0, KT // 8)           # D over AllGather3a's rows
            gather("ag3", "b")

            # s1 transposes + start of out.T while AllGather3b runs
            with tc.tile_pool(name="tps2", bufs=4, space="PSUM") as TPS2:
                for c in range(2):
                    for m in range(8):
                        tp2 = TPS2.tile([128, 128], f32, name="tp2",
                                        tag="tp2")
                        nc.tensor.transpose(
                            tp2[:], s1_sb[m][:, c * 128:(c + 1) * 128],
                            ident_sb[:])
                        nc.vector.tensor_copy(
                            h2T[2 + c][:, m * 128:(m + 1) * 128], tp2[:])
            fstack = ExitStack()
            FPS = fstack.enter_context(tc.tile_pool(name="fps", bufs=1,
                                                    space="PSUM"))
            fq = [FPS.tile([128, H], f32, name=f"fq{i}", tag=f"fq{i}")
                  for i in range(4)]  # i = mo*2+n
            for mo in range(2):
                for n in range(2):
                    for k in range(4):
                        nc.tensor.matmul(
                            fq[mo * 2 + n][:],
                            wf_sb[k][:, mo * 128:(mo + 1) * 128],
                            h2T[k][:, n * H:(n + 1) * H],
                            start=(k == 0), stop=False)
            d_quads(KT // 8, KT // 4)         # D over AllGather3b's rows
            for mo in range(2):
                for n in range(2):
                    nc.vector.tensor_copy(h2T[4 + mo][:, n * H:(n + 1) * H],
                                          psd[mo * 2 + n][:])

            # ========== final tail: add s2b k-tiles + bias, store ============
            for mo in range(2):
                for n in range(2):
                    for k in range(4, H2T):
                        nc.tensor.matmul(
                            fq[mo * 2 + n][:],
                            wf_sb[k][:, mo * 128:(mo + 1) * 128],
                            h2T[k][:, n * H:(n + 1) * H],
                            start=False, stop=(k == H2T - 1))
                    osb = DR.tile([128, H], f32, name="osb", tag="osb")
                    nc.vector.tensor_scalar_add(osb[:], fq[mo * 2 + n][:],
                                                bfT_sb[:, mo:mo + 1])
                    nc.sync.dma_start(
                        outT_d[mo * 128:(mo + 1) * 128, n * H:(n + 1) * H],
                        osb[:])
            fstack.close()

    nc.compile()
    _BUILT["nc"] = nc
    return nc


def _half_major_perm():
    """Slab permutation: k' -> global 128-row slab index, half-major order:
    [r0 rows0:512 | r1 rows0:512 | ... | r7 rows0:512 | r0 rows512:1024...]"""
    perm = []
    for g in range(2):
        for r in range(NC):
            for j in range(4):
                perm.append(r * 8 + g * 4 + j)
    return perm


def prep_in_maps(x, adj, W1, b1, W2, b2, gamma, beta, Wf, bf):
    """Host-side sharding / layout prep. Returns one input dict per core."""
    import ml_dtypes

    x = np.asarray(x, dtype=np.float32)
    adj = np.asarray(adj, dtype=np.float32)
    W1 = np.asarray(W1, dtype=np.float32)
    b1 = np.asarray(b1, dtype=np.float32)
    W2 = np.asarray(W2, dtype=np.float32)
    b2 = np.asarray(b2, dtype=np.float32)
    gamma = np.asarray(gamma, dtype=np.float32)
    beta = np.asarray(beta, dtype=np.float32)
    Wf = np.asarray(Wf, dtype=np.float32)
    bf = np.asarray(bf, dtype=np.float32)

    perm = _half_major_perm()
    row_perm = np.concatenate(
        [np.arange(s * 128, (s + 1) * 128) for s in perm])

    xTp = np.ascontiguousarray(x.T[:, row_perm])         # [128, 8192]
    w1cat = np.ascontiguousarray(
        np.concatenate([W1[1], W1[2]], axis=1))          # [128, 1024]
    b1cat = np.concatenate([b1[1], b1[2]])               # [1024]
    b1bc = np.ascontiguousarray(
        np.broadcast_to(b1cat[None, :], (128, 2 * H)))
    w2cat = np.ascontiguousarray(
        np.concatenate([W2[1], W2[2]], axis=1))          # [1536, 512]
    b2cat = np.concatenate([b2[1], b2[2]])               # [512]
    b2bc = np.ascontiguousarray(
        np.broadcast_to(b2cat[None, :], (128, 2 * OUT)))
    gcol = np.ascontiguousarray(gamma.reshape(HT, 128).T)
    bcol = np.ascontiguousarray(beta.reshape(HT, 128).T)
    ident = np.eye(128, dtype=np.float32)

    shared = {
        "xT": xTp,
        "w1cat": w1cat,
        "w1h0": np.ascontiguousarray(W1[0]),
        "b1bc": b1bc,
        "w2cat": w2cat,
        "w2h0": np.ascontiguousarray(W2[0]),
        "b2bc": b2bc,
        "b2h0T": np.ascontiguousarray(b2[0].reshape(2, 128).T),
        "wf": np.ascontiguousarray(Wf),
        "bfT": np.ascontiguousarray(bf.reshape(2, 128).T),
        "gcol": gcol,
        "bcol": bcol,
        "ident": ident,
    }
    import os as _os
    if _os.environ.get("LDW_EXPERIMENT"):
        shared["cachebust"] = np.zeros((1, 4), np.float32)

    in_maps = []
    for d in range(NC):
        r0, r1 = d * ROWS, (d + 1) * ROWS
        m = dict(shared)
        adjT = adj[r0:r1].T[row_perm]                    # [8192, 1024]
        m["adjT"] = np.ascontiguousarray(adjT.astype(ml_dtypes.bfloat16))
        m["xTloc"] = np.ascontiguousarray(x[r0:r1].T)    # [128, 1024]
        in_maps.append(m)
    return in_maps


def run_on_hw(in_maps, trace=False):
    from concourse import bass_utils
    nc = build_program()
    return bass_utils.run_bass_kernel_spmd(
        nc, in_maps, core_ids=list(range(NC)), trace=trace)


def kernel(x, adj, W1, b1, W2, b2, gamma, beta, Wf, bf):
    in_maps = prep_in_maps(x, adj, W1, b1, W2, b2, gamma, beta, Wf, bf)
    res = run_on_hw(in_maps)
    out = np.concatenate(
        [np.ascontiguousarray(res.results[d]["outT"].T) for d in range(NC)],
        axis=0)
    return out.astype(np.float32)


# revision 31
# speedup vs baseline: 1.0340x; 1.0340x over previous
"""MixHop (2-hop) GNN forward on 8 TRN2 NeuronCores.

Sharding: adj and the output are row-sharded over N=8192 across 8 cores
(1024 rows each); x and all weights are replicated. Each propagation
adj_loc @ v is a local [1024,8192]@[8192,F] matmul; v is produced
row-sharded and AllGathered between hops.

Precision: propagation matmuls (adj-sided, the bulk of bytes+flops) run
in bf16 with fp32 PSUM accumulation; dense-layer transforms and BN run
in fp32r (full-rate reduced fp32). Measured end-to-end relative error
~2e-3.

Orientation notes:
- "natural"   = rows on partitions (needed for AllGather row-concat and
  as the K axis of the next propagation)
- "transposed" = features on partitions (needed as lhsT of the next
  dense layer; makes BatchNorm affine per-partition)
Pass B and D emit transposed outputs directly; pass A's t1 half and
pass C's s1 half are transposed on the PE with an identity matmul.
b1[0] (hop-0 bias of layer 1) is dropped: a per-column constant shift
is exactly cancelled by the training-mode BatchNorm that follows.

Scheduling notes:
- Every AllGather is split into two row-half chunks. Producer passes
  emit their first row-half, trigger chunk-a, and compute the second
  half under it; consumer passes contract chunk-a's rows while chunk-b
  is still gathering. To keep the contraction k-axis contiguous per
  chunk, the host permutes adj's columns (and x's rows) into
  "half-major" order: [r0 rows0:512 | r1 rows0:512 | ... | r0 rows
  512:1024 | ...]. Local row order (outputs) is unpermuted.
- Propagation k-loops process 4 k-slabs per iteration -> ~4.3us
  contiguous matmul bursts, which hold the PE HAM clock-gate at full
  rate (short bursts leave the PE throttled to 1.2 GHz).
- BN stats/AllReduce/normalize for h tiles 0..7 run under pass B; T2's
  first 8 k-tiles accumulate under AllReduce-b so only a short tail
  waits on it. y0.T runs under AllGather2b; s1 transposes run before
  AllGather3.
- DMA loads alternate between the two HWDGE rings (sync/scalar);
  SBUF->DRAM drains go via SWDGE (gpsimd).
"""
import sys
from contextlib import ExitStack

sys.path.insert(0, "/opt/trn_rl_repo")

import numpy as np

N, IN, H, OUT = 8192, 128, 512, 256
NC = 8
ROWS = N // NC          # 1024 rows per core
KT = N // 128           # 64 k-tiles of the propagation contraction
KH = KT // 2            # 32 k-tiles per gather chunk
HT = 3 * H // 128       # 12 feature tiles of h.T
H2T = 3 * OUT // 128    # 6 feature tiles of h2.T
EPS = 1e-5

_BUILT = {}


def build_program():
    """Build and compile the Bass program (cached)."""
    if "nc" in _BUILT:
        return _BUILT["nc"]

    import concourse.bacc as bacc
    import concourse.tile as tile
    import concourse.mybir as mybir
    from concourse.alu_op_type import AluOpType

    f32 = mybir.dt.float32
    f32r = mybir.dt.float32r
    bf16 = mybir.dt.bfloat16
    AF = mybir.ActivationFunctionType
    AX = mybir.AxisListType

    nc = bacc.Bacc("TRN2", target_bir_lowering=False, debug=False,
                   num_devices=NC)

    import os as _os
    if _os.environ.get("LDW_EXPERIMENT"):
        # harmless extra input to change the program hash (cache bust)
        nc.dram_tensor("cachebust", [1, 4], mybir.dt.float32,
                       kind="ExternalInput")

    # ---- external inputs (per-core values supplied by the host) ----
    adjT_d = nc.dram_tensor("adjT", [N, ROWS], bf16, kind="ExternalInput")
    xT_d = nc.dram_tensor("xT", [IN, N], f32r, kind="ExternalInput")
    xTloc_d = nc.dram_tensor("xTloc", [IN, ROWS], f32r, kind="ExternalInput")
    w1cat_d = nc.dram_tensor("w1cat", [IN, 2 * H], f32r, kind="ExternalInput")
    w1h0_d = nc.dram_tensor("w1h0", [IN, H], f32r, kind="ExternalInput")
    b1bc_d = nc.dram_tensor("b1bc", [128, 2 * H], f32, kind="ExternalInput")
    w2cat_d = nc.dram_tensor("w2cat", [3 * H, 2 * OUT], f32r, kind="ExternalInput")
    w2h0_d = nc.dram_tensor("w2h0", [3 * H, OUT], f32r, kind="ExternalInput")
    b2bc_d = nc.dram_tensor("b2bc", [128, 2 * OUT], f32, kind="ExternalInput")
    b2h0T_d = nc.dram_tensor("b2h0T", [128, 2], f32, kind="ExternalInput")
    wf_d = nc.dram_tensor("wf", [3 * OUT, OUT], f32r, kind="ExternalInput")
    bfT_d = nc.dram_tensor("bfT", [128, 2], f32, kind="ExternalInput")
    gcol_d = nc.dram_tensor("gcol", [128, HT], f32, kind="ExternalInput")
    bcol_d = nc.dram_tensor("bcol", [128, HT], f32, kind="ExternalInput")
    ident_d = nc.dram_tensor("ident", [128, 128], f32, kind="ExternalInput")

    outT_d = nc.dram_tensor("outT", [OUT, ROWS], f32, kind="ExternalOutput")

    rg = [list(range(NC))]

    def ring(k):
        return nc.sync if k % 2 == 0 else nc.scalar

    with tile.TileContext(nc) as tc, ExitStack() as st:
        dram = st.enter_context(tc.tile_pool(name="dram", bufs=1, space="DRAM"))
        P = st.enter_context(tc.tile_pool(name="persist", bufs=1))
        DR = st.enter_context(tc.tile_pool(name="drain", bufs=2))

        # ---- DRAM intermediates ----
        v_dram = dram.tile([N, H], bf16, name="v_dram")   # x1 transform only
        agi = {}
        ago = {}
        for nm, rows, cols in (("ag1", ROWS // 2, H), ("ag2", ROWS // 2,
                                                       2 * OUT),
                               ("ag3", ROWS // 2, OUT)):
            for ch in "ab":
                agi[nm + ch] = dram.tile([rows, cols], bf16,
                                         name=f"{nm}{ch}_in")
                ago[nm + ch] = dram.tile([rows * NC, cols], bf16,
                                         name=f"{nm}{ch}_out",
                                         addr_space="Shared")
        ar_a_in = dram.tile([128, 16], f32, name="ar_a_in")
        ar_a_out = dram.tile([128, 16], f32, name="ar_a_out",
                             addr_space="Shared")
        ar_b_in = dram.tile([128, 8], f32, name="ar_b_in")
        ar_b_out = dram.tile([128, 8], f32, name="ar_b_out",
                             addr_space="Shared")

        def gather(nm, ch):
            nc.gpsimd.collective_compute(
                "AllGather", AluOpType.bypass, replica_groups=rg,
                ins=[agi[nm + ch][:].opt()], outs=[ago[nm + ch][:].opt()])

        def gsrc(nm, k):
            """k-slab [128, cols] of the gathered tensor, half-major order."""
            t = ago[nm + ("a" if k < KH else "b")]
            kk = k if k < KH else k - KH
            return t[kk * 128:(kk + 1) * 128, :]

        def gsrc_pair(nm, q):
            """Pair-slab AP [128, 2, cols] for k-slabs 2q, 2q+1."""
            t = ago[nm + ("a" if 2 * q < KH else "b")]
            qq = q if 2 * q < KH else q - KH // 2
            return t[:].rearrange("(a two p) f -> a p two f",
                                  two=2, p=128)[qq]

        # ---- small persistents (to the end) ----
        # w1cat first: it gates the very first matmul of the kernel
        W1P = st.enter_context(tc.tile_pool(name="w1pool", bufs=1))
        w1cat_sb = W1P.tile([IN, 2 * H], f32r, name="w1cat_sb")
        nc.scalar.dma_start(w1cat_sb[:], w1cat_d[:, :])
        b1bc_sb = W1P.tile([128, 2 * H], f32, name="b1bc_sb")
        nc.scalar.dma_start(b1bc_sb[:], b1bc_d[:, :])
        xTloc_sb = P.tile([IN, ROWS], f32r, name="xTloc_sb")
        nc.scalar.dma_start(xTloc_sb[:], xTloc_d[:, :])
        w1h0_sb = P.tile([IN, H], f32r, name="w1h0_sb")
        nc.scalar.dma_start(w1h0_sb[:], w1h0_d[:, :])
        b2h0T_sb = P.tile([128, 2], f32, name="b2h0T_sb")
        nc.scalar.dma_start(b2h0T_sb[:], b2h0T_d[:, :])
        bfT_sb = P.tile([128, 2], f32, name="bfT_sb")
        nc.scalar.dma_start(bfT_sb[:], bfT_d[:, :])
        gcol_sb = P.tile([128, HT], f32, name="gcol_sb")
        nc.scalar.dma_start(gcol_sb[:], gcol_d[:, :])
        bcol_sb = P.tile([128, HT], f32, name="bcol_sb")
        nc.scalar.dma_start(bcol_sb[:], bcol_d[:, :])
        ident_sb = P.tile([128, 128], f32, name="ident_sb")
        nc.scalar.dma_start(ident_sb[:], ident_d[:, :])
        wf_sb = [P.tile([128, OUT], f32r, name=f"wf{k}") for k in range(H2T)]
        for k in range(H2T):
            nc.scalar.dma_start(wf_sb[k][:], wf_d[k * 128:(k + 1) * 128, :])
        sumc = P.tile([128, HT], f32, name="sumc")
        sqc = P.tile([128, HT], f32, name="sqc")
        scale_c = P.tile([128, HT], f32, name="scale_c")
        shift_c = P.tile([128, HT], f32, name="shift_c")
        stat_a = P.tile([128, 16], f32, name="stat_a")
        stat_b = P.tile([128, 8], f32, name="stat_b")
        # h2.T (fp32r): tiles 0-1 y0.T, 2-3 s1.T, 4-5 s2b.T
        h2T = [P.tile([128, ROWS], f32r, name=f"h2T{t}") for t in range(H2T)]

        def bn_affine(stat, lo, hi):
            """Compute scale/shift columns [lo,hi) from gathered stats."""
            w = hi - lo
            mu = DR.tile([128, w], f32, name="mu", tag=f"mu{lo}")
            nc.vector.tensor_scalar_mul(mu[:], stat[:, :w], 1.0 / N)
            ex2 = DR.tile([128, w], f32, name="ex2", tag=f"ex2{lo}")
            nc.vector.tensor_scalar(ex2[:], stat[:, w:2 * w], 1.0 / N, EPS,
                                    AluOpType.mult, AluOpType.add)
            var = DR.tile([128, w], f32, name="var", tag=f"var{lo}")
            nc.vector.scalar_tensor_tensor(var[:], mu[:], -1.0, mu[:],
                                           AluOpType.mult, AluOpType.mult)
            nc.vector.tensor_add(var[:], var[:], ex2[:])
            std = DR.tile([128, w], f32, name="std", tag=f"std{lo}")
            nc.scalar.activation(std[:], var[:], AF.Sqrt)
            rstd = DR.tile([128, w], f32, name="rstd", tag=f"rstd{lo}")
            nc.vector.reciprocal(rstd[:], std[:])
            nc.vector.tensor_mul(scale_c[:, lo:hi], gcol_sb[:, lo:hi], rstd[:])
            nc.vector.scalar_tensor_tensor(shift_c[:, lo:hi], mu[:], -1.0,
                                           scale_c[:, lo:hi],
                                           AluOpType.mult, AluOpType.mult)
            nc.vector.tensor_add(shift_c[:, lo:hi], shift_c[:, lo:hi],
                                 bcol_sb[:, lo:hi])

        def stats_for(tiles, sq_pool, tag):
            for t in tiles:
                nc.vector.reduce_sum(sumc[:, t:t + 1], hT[t][:], axis=AX.X)
                sq = sq_pool.tile([128, ROWS], f32, name=tag, tag=tag)
                nc.vector.scalar_tensor_tensor(
                    sq[:], hT[t][:], 1.0, hT[t][:],
                    AluOpType.mult, AluOpType.mult,
                    accum_out=sqc[:, t:t + 1])

        # ============ T1 + pass A (t2a), pipelined ===========================
        with (
            tc.tile_pool(name="v2pool", bufs=1) as V2P,
            tc.tile_pool(name="xkpool", bufs=1) as XKP,
        ):
            v2sb = [V2P.tile([128, H], bf16, name=f"v2s{k}")
                    for k in range(KT)]
            xk_sb = [XKP.tile([128, 128], f32r, name=f"xk{k}")
                     for k in range(KT)]

            # T1-x2: v2 = x @ W1[2] + b1[2]  (SBUF-resident, bf16)
            with tc.tile_pool(name="t1ps2", bufs=3, space="PSUM") as T1PS2:
                for k in range(KT):
                    nc.gpsimd.dma_start(xk_sb[k][:],
                                        xT_d[:, k * 128:(k + 1) * 128])
                    vps2 = T1PS2.tile([128, H], f32, name="vps2", tag="vps2")
                    nc.tensor.matmul(vps2[:], xk_sb[k][:],
                                     w1cat_sb[:, H:2 * H],
                                     start=True, stop=True)
                    nc.vector.tensor_tensor(v2sb[k][:], vps2[:],
                                            b1bc_sb[:, H:2 * H],
                                            AluOpType.add)

            # A-t2a group 0 with T1-x1 interleaved into its dense PE stream
            with (
                tc.tile_pool(name="ag0slab", bufs=2) as AS,
                tc.tile_pool(name="ag0ps", bufs=1, space="PSUM") as APS,
                tc.tile_pool(name="t1ps1", bufs=2, space="PSUM") as T1PS1,
            ):
                acc = [APS.tile([128, H], f32, name=f"acc0{m}",
                                tag=f"acc0{m}") for m in range(4)]
                for q in range(KT // 4):
                    slabs = []
                    for t in range(4):
                        k = 4 * q + t
                        asl = AS.tile([128, H], bf16, name=f"asl{t}",
                                      tag=f"asl{t}")
                        ring(t).dma_start(asl[:],
                                          adjT_d[k * 128:(k + 1) * 128, 0:H])
                        slabs.append(asl)
                    for t in range(4):
                        k = 4 * q + t
                        for m in range(4):
                            nc.tensor.matmul(
                                acc[m][:], slabs[t][:, m * 128:(m + 1) * 128],
                                v2sb[k][:], start=(k == 0),
                                stop=(k == KT - 1))
                    # T1-x1 for 4 slabs, slotted into the same stream
                    for t in range(4):
                        k = 4 * q + t
                        vps1 = T1PS1.tile([128, H], f32, name="vps1",
                                          tag="vps1")
                        nc.tensor.matmul(vps1[:], xk_sb[k][:],
                                         w1cat_sb[:, 0:H],
                                         start=True, stop=True)
                        v1sb = DR.tile([128, H], bf16, name="v1sb",
                                       tag="v1sb")
                        nc.vector.tensor_tensor(v1sb[:], vps1[:],
                                                b1bc_sb[:, 0:H],
                                                AluOpType.add)
                        nc.gpsimd.dma_start(
                            v_dram[k * 128:(k + 1) * 128, :], v1sb[:])
                for m in range(4):
                    t2a = DR.tile([128, H], bf16, name="t2a", tag="t2a")
                    nc.vector.tensor_copy(t2a[:], acc[m][:])
                    nc.gpsimd.dma_start(
                        agi["ag1a"][m * 128:(m + 1) * 128, :], t2a[:])

            gather("ag1", "a")

            # A-t2a group 1
            with (
                tc.tile_pool(name="ag1slab", bufs=2) as AS1,
                tc.tile_pool(name="ag1ps", bufs=1, space="PSUM") as APS1,
            ):
                acc1 = [APS1.tile([128, H], f32, name=f"acc1{m}",
                                  tag=f"acc1{m}") for m in range(4)]
                for q in range(KT // 4):
                    slabs = []
                    for t in range(4):
                        k = 4 * q + t
                        asl = AS1.tile([128, H], bf16, name=f"bsl{t}",
                                       tag=f"bsl{t}")
                        ring(t).dma_start(asl[:],
                                          adjT_d[k * 128:(k + 1) * 128,
                                                 H:ROWS])
                        slabs.append(asl)
                    for t in range(4):
                        k = 4 * q + t
                        for m in range(4):
                            nc.tensor.matmul(
                                acc1[m][:], slabs[t][:, m * 128:(m + 1) * 128],
                                v2sb[k][:], start=(k == 0),
                                stop=(k == KT - 1))
                for m in range(4):
                    t2a = DR.tile([128, H], bf16, name="t2a", tag="t2a")
                    nc.vector.tensor_copy(t2a[:], acc1[m][:])
                    nc.gpsimd.dma_start(
                        agi["ag1b"][m * 128:(m + 1) * 128, :], t2a[:])

            gather("ag1", "b")

        # hT in fp32r: tiles 0-3 hop0.T, 4-7 t1.T, 8-11 t2b.T.
        PH = st.enter_context(tc.tile_pool(name="hpool", bufs=1))
        hT = [PH.tile([128, ROWS], f32r, name=f"hT{t}") for t in range(HT)]

        # ========== hop0.T + its stats (runs under AllGather1) ===============
        with (
            tc.tile_pool(name="h0ps", bufs=2, space="PSUM") as H0PS,
            tc.tile_pool(name="sqps0", bufs=1, space="PSUM") as SQPS0,
        ):
            for mo in range(4):
                for n in range(2):
                    h0ps = H0PS.tile([128, H], f32, name="h0ps", tag="h0ps")
                    nc.tensor.matmul(h0ps[:],
                                     w1h0_sb[:, mo * 128:(mo + 1) * 128],
                                     xTloc_sb[:, n * H:(n + 1) * H],
                                     start=True, stop=True)
                    nc.vector.tensor_copy(hT[mo][:, n * H:(n + 1) * H],
                                          h0ps[:])
            stats_for(range(4), SQPS0, "sq0")

        # t1 natural (fp32), transposed as soon as each row-group lands
        PT1 = st.enter_context(tc.tile_pool(name="t1nat", bufs=1))
        t1_sb = [PT1.tile([128, H], f32, name=f"t1n{m}") for m in range(8)]

        # ========= A-t1 row-groups + transposes ==============================
        for g in range(2):
            with (
                tc.tile_pool(name=f"a2slab{g}", bufs=2) as AS2,
                tc.tile_pool(name=f"aps2{g}", bufs=1, space="PSUM") as APS2,
            ):
                acc2 = [APS2.tile([128, H], f32, name=f"ac2{g}{m}",
                                  tag=f"ac2{g}{m}") for m in range(4)]
                for q in range(KT // 4):
                    slabs = []
                    for t in range(4):
                        k = 4 * q + t
                        asl = AS2.tile([128, H], bf16, name=f"a2s{t}",
                                       tag=f"a2s{t}")
                        ring(t).dma_start(
                            asl[:], adjT_d[k * 128:(k + 1) * 128,
                                           g * H:(g + 1) * H])
                        slabs.append(asl)
                    v1p = AS2.tile([128, 2 * H], bf16, name="v1p", tag="v1p")
                    nc.scalar.dma_start(
                        v1p[:].rearrange("p (two f) -> p two f", two=2),
                        v_dram[:].rearrange("(a two p) f -> a p two f",
                                            two=2, p=128)[2 * q])
                    v1p2 = AS2.tile([128, 2 * H], bf16, name="v1p2",
                                    tag="v1p2")
                    nc.gpsimd.dma_start(
                        v1p2[:].rearrange("p (two f) -> p two f", two=2),
                        v_dram[:].rearrange("(a two p) f -> a p two f",
                                            two=2, p=128)[2 * q + 1])
                    vv = [v1p[:, 0:H], v1p[:, H:2 * H],
                          v1p2[:, 0:H], v1p2[:, H:2 * H]]
                    for t in range(4):
                        k = 4 * q + t
                        for m in range(4):
                            nc.tensor.matmul(
                                acc2[m][:],
                                slabs[t][:, m * 128:(m + 1) * 128],
                                vv[t], start=(k == 0), stop=(k == KT - 1))
                for m in range(4):
                    nc.vector.tensor_copy(t1_sb[4 * g + m][:], acc2[m][:])
            with tc.tile_pool(name=f"tps{g}", bufs=4, space="PSUM") as TPS:
                for c in range(4):
                    for m in range(4 * g, 4 * g + 4):
                        tp = TPS.tile([128, 128], f32, name="tp", tag="tp")
                        nc.tensor.transpose(
                            tp[:], t1_sb[m][:, c * 128:(c + 1) * 128],
                            ident_sb[:])
                        nc.vector.tensor_copy(
                            hT[4 + c][:, m * 128:(m + 1) * 128], tp[:])

        # ========= t1 stats + AllReduce-a + normalize tiles 0..7 =============
        with tc.tile_pool(name="sqps1", bufs=1, space="PSUM") as SQPS1:
            stats_for(range(4, 8), SQPS1, "sq1")
        nc.gpsimd.dma_start(ar_a_in[:, 0:8], sumc[:, 0:8])
        nc.gpsimd.dma_start(ar_a_in[:, 8:16], sqc[:, 0:8])
        nc.gpsimd.collective_compute(
            "AllReduce", AluOpType.add, replica_groups=rg,
            ins=[ar_a_in[:].opt()], outs=[ar_a_out[:].opt()])
        nc.sync.dma_start(stat_a[:], ar_a_out[:, :])
        bn_affine(stat_a, 0, 8)
        for t in range(8):
            nc.scalar.activation(hT[t][:], hT[t][:], AF.Relu,
                                 bias=shift_c[:, t:t + 1],
                                 scale=scale_c[:, t:t + 1])

        # layer-2 weights, loaded during pass B
        PW2 = st.enter_context(tc.tile_pool(name="w2pool", bufs=1))
        w2cat_sb = [PW2.tile([128, 2 * OUT], f32r, name=f"w2cat{k}")
                    for k in range(HT)]
        for k in range(HT):
            nc.scalar.dma_start(w2cat_sb[k][:],
                                w2cat_d[k * 128:(k + 1) * 128, :])
        w2h0_sb = [PW2.tile([128, OUT], f32r, name=f"w2h0{k}")
                   for k in range(HT)]
        for k in range(HT):
            nc.scalar.dma_start(w2h0_sb[k][:],
                                w2h0_d[k * 128:(k + 1) * 128, :])
        b2bc_sb = PW2.tile([128, 2 * OUT], f32, name="b2bc_sb")
        nc.scalar.dma_start(b2bc_sb[:], b2bc_d[:, :])

        # ================= B: t2b.T = (adj_loc @ t2a_full).T =================
        with (
            tc.tile_pool(name="bslabs", bufs=2) as BS,
            tc.tile_pool(name="bps", bufs=1, space="PSUM") as BPS,
        ):
            psb = [BPS.tile([128, H], f32, name=f"psb{i}", tag=f"psb{i}")
                   for i in range(8)]  # i = mo*2+n
            for q in range(KT // 2):
                aslab = BS.tile([128, ROWS], bf16, name="aslab", tag="aslab")
                nc.sync.dma_start(aslab[:],
                                  adjT_d[2 * q * 128:(2 * q + 1) * 128, :])
                aslab2 = BS.tile([128, ROWS], bf16, name="aslab2",
                                 tag="aslab2")
                nc.scalar.dma_start(
                    aslab2[:], adjT_d[(2 * q + 1) * 128:(2 * q + 2) * 128, :])
                tsp = BS.tile([128, 2 * H], bf16, name="tsp", tag="tsp")
                nc.gpsimd.dma_start(
                    tsp[:].rearrange("p (two f) -> p two f", two=2),
                    gsrc_pair("ag1", q))
                for t, asl in ((0, aslab), (1, aslab2)):
                    k = 2 * q + t
                    for mo in range(4):
                        for n in range(2):
                            nc.tensor.matmul(
                                psb[mo * 2 + n][:],
                                tsp[:, t * H + mo * 128:
                                    t * H + (mo + 1) * 128],
                                asl[:, n * H:(n + 1) * H],
                                start=(k == 0), stop=(k == KT - 1))
            for mo in range(4):
                for n in range(2):
                    nc.vector.tensor_copy(hT[8 + mo][:, n * H:(n + 1) * H],
                                          psb[mo * 2 + n][:])

        # ========== stats for t2b + AllReduce-b ==============================
        with tc.tile_pool(name="sqps2", bufs=1, space="PSUM") as SQPS2:
            stats_for(range(8, HT), SQPS2, "sq2")
        nc.gpsimd.dma_start(ar_b_in[:, 0:4], sumc[:, 8:12])
        nc.gpsimd.dma_start(ar_b_in[:, 4:8], sqc[:, 8:12])
        nc.gpsimd.collective_compute(
            "AllReduce", AluOpType.add, replica_groups=rg,
            ins=[ar_b_in[:].opt()], outs=[ar_b_out[:].opt()])

        # ========== T2 phase 1 (k=0..7) under AllReduce-b ====================
        with tc.tile_pool(name="ypsb", bufs=1, space="PSUM") as YPSb:
            ypss = [None] * 8
            for m in range(4, 8):
                ypss[m] = YPSb.tile([128, 2 * OUT], f32, name=f"ypsb{m}",
                                    tag=f"ypsb{m}")

            def t2_phase2(half):
                for m in range(4 * half, 4 * half + 4):
                    for k in range(8, HT):
                        nc.tensor.matmul(ypss[m][:],
                                         hT[k][:, m * 128:(m + 1) * 128],
                                         w2cat_sb[k][:],
                                         start=False, stop=(k == HT - 1))
                    ysb = DR.tile([128, 2 * OUT], bf16, name="ysb", tag="ysb")
                    nc.vector.tensor_tensor(ysb[:], ypss[m][:], b2bc_sb[:],
                                            AluOpType.add)
                    nc.gpsimd.dma_start(
                        agi["ag2" + "ab"[half]][
                            (m - 4 * half) * 128:(m - 4 * half + 1) * 128, :],
                        ysb[:])
                gather("ag2", "ab"[half])

            with tc.tile_pool(name="ypsa", bufs=1, space="PSUM") as YPSa:
                for m in range(4):
                    ypss[m] = YPSa.tile([128, 2 * OUT], f32, name=f"ypsa{m}",
                                        tag=f"ypsa{m}")
                for m in range(8):
                    for k in range(8):
                        nc.tensor.matmul(ypss[m][:],
                                         hT[k][:, m * 128:(m + 1) * 128],
                                         w2cat_sb[k][:],
                                         start=(k == 0), stop=False)

                # AllReduce-b lands: finish BN for t2b tiles
                nc.sync.dma_start(stat_b[:], ar_b_out[:, :])
                bn_affine(stat_b, 8, HT)
                for t in range(8, HT):
                    nc.scalar.activation(hT[t][:], hT[t][:], AF.Relu,
                                         bias=shift_c[:, t:t + 1],
                                         scale=scale_c[:, t:t + 1])
                t2_phase2(0)

            # YPSa closed: its banks free for y0T, which fills the PE while
            # AllGather2a runs; phase 2b follows.
            with tc.tile_pool(name="y0ps", bufs=2, space="PSUM") as Y0PS:
                for mo in range(2):
                    for n in range(2):
                        y0ps = Y0PS.tile([128, H], f32, name="y0ps",
                                         tag="y0ps")
                        for k in range(HT):
                            nc.tensor.matmul(
                                y0ps[:],
                                w2h0_sb[k][:, mo * 128:(mo + 1) * 128],
                                hT[k][:, n * H:(n + 1) * H],
                                start=(k == 0), stop=(k == HT - 1))
                        nc.vector.tensor_scalar_add(
                            h2T[mo][:, n * H:(n + 1) * H], y0ps[:],
                            b2h0T_sb[:, mo:mo + 1])
                t2_phase2(1)

        # s1 natural (fp32), transposed before AllGather3 completes
        PS1 = st.enter_context(tc.tile_pool(name="s1nat", bufs=1))
        s1_sb = [PS1.tile([128, OUT], f32, name=f"s1n{m}") for m in range(8)]

        # ========== C: [s1|s2a] = adj_loc @ [y1|y2] (natural) ================
        with tc.tile_pool(name="cpsb", bufs=1, space="PSUM") as CPSb:
            pscb = [CPSb.tile([128, 2 * OUT], f32, name=f"pscb{m}",
                              tag=f"pscb{m}") for m in range(4, 8)]
            with (
                tc.tile_pool(name="cslabs", bufs=2) as CS,
                tc.tile_pool(name="cpsa", bufs=1, space="PSUM") as CPSa,
            ):
                psc = [CPSa.tile([128, 2 * OUT], f32, name=f"psca{m}",
                                 tag=f"psca{m}") for m in range(4)] + pscb
                for q in range(KT // 2):
                    aslab = CS.tile([128, ROWS], bf16, name="aslab",
                                    tag="aslab")
                    nc.sync.dma_start(
                        aslab[:], adjT_d[2 * q * 128:(2 * q + 1) * 128, :])
                    aslab2 = CS.tile([128, ROWS], bf16, name="aslab2",
                                     tag="aslab2")
                    nc.scalar.dma_start(
                        aslab2[:],
                        adjT_d[(2 * q + 1) * 128:(2 * q + 2) * 128, :])
                    ysp = CS.tile([128, 4 * OUT], bf16, name="ysp", tag="ysp")
                    nc.sync.dma_start(
                        ysp[:].rearrange("p (two f) -> p two f", two=2),
                        gsrc_pair("ag2", q))
                    for t, asl in ((0, aslab), (1, aslab2)):
                        for m in range(8):
                            nc.tensor.matmul(
                                psc[m][:], asl[:, m * 128:(m + 1) * 128],
                                ysp[:, t * 2 * OUT:(t + 1) * 2 * OUT],
                                start=(2 * q + t == 0),
                                stop=(2 * q + t == KT - 1))
                # drains half a + AllGather3a
                for m in range(4):
                    nc.vector.tensor_copy(s1_sb[m][:], psc[m][:, :OUT])
                    s2a = DR.tile([128, OUT], bf16, name="s2a", tag="s2a")
                    nc.vector.tensor_copy(s2a[:], psc[m][:, OUT:])
                    nc.gpsimd.dma_start(
                        agi["ag3a"][m * 128:(m + 1) * 128, :], s2a[:])
                gather("ag3", "a")
            # CPSa closed: transposes for half a run during half-b drains
            with tc.tile_pool(name="tps2a", bufs=4, space="PSUM") as TPS2a:
                for m in range(4, 8):
                    nc.vector.tensor_copy(s1_sb[m][:], pscb[m - 4][:, :OUT])
                    s2a = DR.tile([128, OUT], bf16, name="s2a", tag="s2a")
                    nc.vector.tensor_copy(s2a[:], pscb[m - 4][:, OUT:])
                    nc.gpsimd.dma_start(
                        agi["ag3b"][(m - 4) * 128:(m - 3) * 128, :], s2a[:])
                for c in range(2):
                    for m in range(4):
                        tp2 = TPS2a.tile([128, 128], f32, name="tp2a",
                                         tag="tp2a")
                        nc.tensor.transpose(
                            tp2[:], s1_sb[m][:, c * 128:(c + 1) * 128],
                            ident_sb[:])
                        nc.vector.tensor_copy(
                            h2T[2 + c][:, m * 128:(m + 1) * 128], tp2[:])
                gather("ag3", "b")

        # ========== s1 transposes (half b, under AllGather3) + D + final =====
        with (
            tc.tile_pool(name="dslabs", bufs=2) as DS,
            tc.tile_pool(name="dps", bufs=1, space="PSUM") as DPS,
        ):
            with tc.tile_pool(name="tps2", bufs=4, space="PSUM") as TPS2:
                for c in range(2):
                    for m in range(4, 8):
                        tp2 = TPS2.tile([128, 128], f32, name="tp2",
                                        tag="tp2")
                        nc.tensor.transpose(
                            tp2[:], s1_sb[m][:, c * 128:(c + 1) * 128],
                            ident_sb[:])
                        nc.vector.tensor_copy(
                            h2T[2 + c][:, m * 128:(m + 1) * 128], tp2[:])
            # final partial: y0/s1 k-tiles of out.T accumulate during the
            # AllGather3 window; the s2b tiles are added after pass D.
            fstack = ExitStack()
            FPS = fstack.enter_context(tc.tile_pool(name="fps", bufs=1,
                                                    space="PSUM"))
            fq = [FPS.tile([128, H], f32, name=f"fq{i}", tag=f"fq{i}")
                  for i in range(4)]  # i = mo*2+n
            for mo in range(2):
                for n in range(2):
                    for k in range(4):
                        nc.tensor.matmul(
                            fq[mo * 2 + n][:],
                            wf_sb[k][:, mo * 128:(mo + 1) * 128],
                            h2T[k][:, n * H:(n + 1) * H],
                            start=(k == 0), stop=False)
            # D: s2b.T = (adj_loc @ s2a_full).T; 4 k-slabs per iteration
            psd = [DPS.tile([128, H], f32, name=f"psd{i}", tag=f"psd{i}")
                   for i in range(4)]  # i = mo*2+n
            for q in range(KT // 4):
                slabs = []
                for t in range(4):
                    k = 4 * q + t
                    asl = DS.tile([128, ROWS], bf16, name=f"dsl{t}",
                                  tag=f"dsl{t}")
                    ring(t).dma_start(asl[:],
                                      adjT_d[k * 128:(k + 1) * 128, :])
                    slabs.append(asl)
                sp1 = DS.tile([128, 2 * OUT], bf16, name="sp1", tag="sp1")
                nc.sync.dma_start(
                    sp1[:].rearrange("p (two f) -> p two f", two=2),
                    gsrc_pair("ag3", 2 * q))
                sp2 = DS.tile([128, 2 * OUT], bf16, name="sp2", tag="sp2")
                nc.gpsimd.dma_start(
                    sp2[:].rearrange("p (two f) -> p two f", two=2),
                    gsrc_pair("ag3", 2 * q + 1))
                ss = [sp1[:, 0:OUT], sp1[:, OUT:2 * OUT],
                      sp2[:, 0:OUT], sp2[:, OUT:2 * OUT]]
                for t in range(4):
                    k = 4 * q + t
                    for mo in range(2):
                        for n in range(2):
                            nc.tensor.matmul(
                                psd[mo * 2 + n][:],
                                ss[t][:, mo * 128:(mo + 1) * 128],
                                slabs[t][:, n * H:(n + 1) * H],
                                start=(k == 0), stop=(k == KT - 1))
            for mo in range(2):
                for n in range(2):
                    nc.vector.tensor_copy(h2T[4 + mo][:, n * H:(n + 1) * H],
                                          psd[mo * 2 + n][:])

            # ========== final tail: add s2b k-tiles + bias, store ============
            for mo in range(2):
                for n in range(2):
                    for k in range(4, H2T):
                        nc.tensor.matmul(
                            fq[mo * 2 + n][:],
                            wf_sb[k][:, mo * 128:(mo + 1) * 128],
                            h2T[k][:, n * H:(n + 1) * H],
                            start=False, stop=(k == H2T - 1))
                    osb = DR.tile([128, H], f32, name="osb", tag="osb")
                    nc.vector.tensor_scalar_add(osb[:], fq[mo * 2 + n][:],
                                                bfT_sb[:, mo:mo + 1])
                    nc.sync.dma_start(
                        outT_d[mo * 128:(mo + 1) * 128, n * H:(n + 1) * H],
                        osb[:])
            fstack.close()

    nc.compile()
    _BUILT["nc"] = nc
    return nc


def _half_major_perm():
    """Slab permutation: k' -> global 128-row slab index, half-major order:
    [r0 rows0:512 | r1 rows0:512 | ... | r7 rows0:512 | r0 rows512:1024...]"""
    perm = []
    for g in range(2):
        for r in range(NC):
            for j in range(4):
                perm.append(r * 8 + g * 4 + j)
    return perm


def prep_in_maps(x, adj, W1, b1, W2, b2, gamma, beta, Wf, bf):
    """Host-side sharding / layout prep. Returns one input dict per core."""
    import ml_dtypes

    x = np.asarray(x, dtype=np.float32)
    adj = np.asarray(adj, dtype=np.float32)
    W1 = np.asarray(W1, dtype=np.float32)
    b1 = np.asarray(b1, dtype=np.float32)
    W2 = np.asarray(W2, dtype=np.float32)
    b2 = np.asarray(b2, dtype=np.float32)
    gamma = np.asarray(gamma, dtype=np.float32)
    beta = np.asarray(beta, dtype=np.float32)
    Wf = np.asarray(Wf, dtype=np.float32)
    bf = np.asarray(bf, dtype=np.float32)

    perm = _half_major_perm()
    row_perm = np.concatenate(
        [np.arange(s * 128, (s + 1) * 128) for s in perm])

    xTp = np.ascontiguousarray(x.T[:, row_perm])         # [128, 8192]
    w1cat = np.ascontiguousarray(
        np.concatenate([W1[1], W1[2]], axis=1))          # [128, 1024]
    b1cat = np.concatenate([b1[1], b1[2]])               # [1024]
    b1bc = np.ascontiguousarray(
        np.broadcast_to(b1cat[None, :], (128, 2 * H)))
    w2cat = np.ascontiguousarray(
        np.concatenate([W2[1], W2[2]], axis=1))          # [1536, 512]
    b2cat = np.concatenate([b2[1], b2[2]])               # [512]
    b2bc = np.ascontiguousarray(
        np.broadcast_to(b2cat[None, :], (128, 2 * OUT)))
    gcol = np.ascontiguousarray(gamma.reshape(HT, 128).T)
    bcol = np.ascontiguousarray(beta.reshape(HT, 128).T)
    ident = np.eye(128, dtype=np.float32)

    shared = {
        "xT": xTp,
        "w1cat": w1cat,
        "w1h0": np.ascontiguousarray(W1[0]),
        "b1bc": b1bc,
        "w2cat": w2cat,
        "w2h0": np.ascontiguousarray(W2[0]),
        "b2bc": b2bc,
        "b2h0T": np.ascontiguousarray(b2[0].reshape(2, 128).T),
        "wf": np.ascontiguousarray(Wf),
        "bfT": np.ascontiguousarray(bf.reshape(2, 128).T),
        "gcol": gcol,
        "bcol": bcol,
        "ident": ident,
    }
    import os as _os
    if _os.environ.get("LDW_EXPERIMENT"):
        shared["cachebust"] = np.zeros((1, 4), np.float32)

    in_maps = []
    for d in range(NC):
        r0, r1 = d * ROWS, (d + 1) * ROWS
        m = dict(shared)
        adjT = adj[r0:r1].T[row_perm]                    # [8192, 1024]
        m["adjT"] = np.ascontiguousarray(adjT.astype(ml_dtypes.bfloat16))
        m["xTloc"] = np.ascontiguousarray(x[r0:r1].T)    # [128, 1024]
        in_maps.append(m)
    return in_maps


def run_on_hw(in_maps, trace=False):
    from concourse import bass_utils
    nc = build_program()
    return bass_utils.run_bass_kernel_spmd(
        nc, in_maps, core_ids=list(range(NC)), trace=trace)


def kernel(x, adj, W1, b1, W2, b2, gamma, beta, Wf, bf):
    in_maps = prep_in_maps(x, adj, W1, b1, W2, b2, gamma, beta, Wf, bf)
    res = run_on_hw(in_maps)
    out = np.concatenate(
        [np.ascontiguousarray(res.results[d]["outT"].T) for d in range(NC)],
        axis=0)
    return out.astype(np.float32)
